# revision 1
# baseline (speedup 1.0000x reference)
"""Trainium2 Bass kernel for the 2-layer dependency-relation GCN (8 cores).

Math per layer l, token i:
    out[i] = relu( W_self[l] @ x[i] + b_self[l]
                   + sum_{e: dep[e]==i} (W_rel[l, rel[e]]   @ x[gov[e]] + b_rel[l, rel[e]])
                   + sum_{e: gov[e]==i} (W_rel[l, R+rel[e]] @ x[dep[e]] + b_rel[l, R+rel[e]]) )
final:  y = h @ W_ff.T + b_ff

Relation-sharded message passing (5 of the 40 directed relations per core),
restructured (v2) for collective overlap:
  * message tiles are ordered by (dest-half, relation-slot); each half's
    AllToAll is triggered as soon as that half's scatters land, so A2A wire
    time overlaps the other half's GEMMs / the accumulation.
  * all collectives ride the Sync engine queue (its only job), so a blocking
    trigger can never stall compute engines.  Send/recv images are separate
    DRAM tensors per (layer, half) => no false WAR serialization.
  * wire images are zeroed during startup (vector queue), not per layer.
  * layer-0 accumulation also PE-transposes each h sub-block (identity
    matmul) and immediately computes the transposed layer-1 self+bias tiles;
    no DRAM round-trip gathers for h^T.
  * layer-1 accumulation runs fully transposed (PSUM holds out^T), so the
    ReLU'd h2^T feeds the FF GEMM directly -- the FF layer is fused into the
    accumulation with zero gathers.
  * self+bias is folded into the PSUM accumulation via an identity matmul;
    ReLU runs on the scalar (Act) engine.
Numerics: bf16 matmul inputs / wire, fp32 PSUM accumulation.
"""

import numpy as np
import ml_dtypes

import concourse.bass as bass
import concourse.mybir as mybir
import concourse.tile as tile
from concourse import bacc
from concourse.bass_utils import run_bass_kernel_spmd

N = 8192
D = 512
R = 20
TWO_R = 2 * R
L = 2
OUT = 256
P = 128
NCORES = 8
RPC = TWO_R // NCORES    # 5 relations per core
BLK = N // NCORES        # 1024 tokens per core
NSUB = BLK // P          # 8 sub-blocks of 128 tokens
HSUB = NSUB // 2
KC = D // P              # 4 contraction chunks
CH_IDX = 2 * P           # idxs per transposing-gather chunk
SC_TILES = 4             # GEMM tiles per scatter-add group

BF16 = ml_dtypes.bfloat16

LAST_EXEC_TIME_NS = None
LAST_RESULTS = None

_CACHE = {}


def _pack_idx16(idx: np.ndarray) -> np.ndarray:
    Ln = len(idx)
    assert Ln % 16 == 0
    base = idx.astype(np.int16).reshape(Ln // 16, 16).T
    return np.tile(base, (8, 1)).copy()


def _plan(dep_idx: np.ndarray, rel_idx: np.ndarray, gov_idx: np.ndarray):
    dep = dep_idx.astype(np.int64)
    gov = gov_idx.astype(np.int64)
    rel = rel_idx.astype(np.int64)

    dest = np.concatenate([dep, gov])
    src = np.concatenate([gov, dep])
    r2 = np.concatenate([rel, rel + R])

    owner = r2 // RPC
    slot = r2 % RPC
    peer = dest // BLK
    sub = (dest % BLK) // P
    half = sub // HSUB
    ksub = sub % HSUB                 # sub index within the half

    # GEMM tiling: tiles per (dest-half, relation-slot), max over cores
    tps = np.ones((2, RPC), dtype=np.int64)
    for h in range(2):
        for s in range(RPC):
            for c in range(NCORES):
                n = int(((owner == c) & (half == h) & (slot == s)).sum())
                tps[h, s] = max(tps[h, s], (n + P - 1) // P)
    tile_slot = []
    tile_off = np.zeros((2, RPC), dtype=np.int64)
    off = 0
    for h in range(2):
        for s in range(RPC):
            tile_off[h, s] = off
            tile_slot.extend([s] * int(tps[h, s]))
            off += int(tps[h, s])
    MT = off
    MTH = [int(tps[0].sum()), int(tps[1].sum())]
    NMSG = MT * P

    # wire layout per half for peer p: [ksub 0..HSUB-1][R1M rows] ++ [OVER]
    cnt = np.zeros((NCORES, NCORES, NSUB), dtype=np.int64)
    np.add.at(cnt, (owner, peer, sub), 1)

    def over_for(r1m):
        ov = 0
        for c in range(NCORES):
            for p in range(NCORES):
                for hh in range(2):
                    tot = sum(
                        max(0, int(cnt[c, p, hh * HSUB + kl]) - r1m)
                        for kl in range(HSUB)
                    )
                    ov = max(ov, tot)
        return int(np.ceil(ov / 16) * 16) if ov else 0

    best = None
    for r1m in (16, 32, 48, 64):
        ov = over_for(r1m)
        seg = HSUB * r1m + ov
        chunks = (NCORES * r1m) // P + (NCORES * ov) // P
        key = (seg, chunks)
        if best is None or key < best[0]:
            best = (key, r1m, ov, seg)
    _, R1M, OVER, SEG = best
    if OVER == 0:
        OVER = 16
        SEG = HSUB * R1M + OVER
    SENDH = NCORES * SEG              # wire rows per half per rank
    J2M = NCORES * R1M // P
    J2O = NCORES * OVER // P
    assert (NCORES * R1M) % P == 0 and (NCORES * OVER) % P == 0
    NCHUNK = NSUB * (J2M + J2O)

    # per-message assignment; send_slot is WITHIN its half's image
    msg_row = np.zeros(2 * N, dtype=np.int64)
    send_slot = np.zeros(2 * N, dtype=np.int64)
    of_pos = np.zeros(2 * N, dtype=np.int64) - 1
    fills = []
    for c in range(NCORES):
        cm = np.nonzero(owner == c)[0]
        fill = np.zeros((2, RPC), dtype=np.int64)
        rfill = np.zeros((NCORES, NSUB), dtype=np.int64)
        ofill = np.zeros((NCORES, 2), dtype=np.int64)
        for m in cm:
            hh = half[m]
            sl = slot[m]
            msg_row[m] = tile_off[hh, sl] * P + fill[hh, sl]
            fill[hh, sl] += 1
            p = peer[m]
            pos = rfill[p, sub[m]]
            rfill[p, sub[m]] += 1
            base = p * SEG
            if pos < R1M:
                send_slot[m] = base + ksub[m] * R1M + pos
            else:
                op_ = ofill[p, hh]
                assert op_ < OVER
                ofill[p, hh] += 1
                of_pos[m] = op_
                send_slot[m] = base + HSUB * R1M + op_
        fills.append(fill)

    # half row/tile ranges + scatter groups (never crossing a half), with
    # trailing-pad trimmed num_idxs per group (max over cores, 16-aligned)
    halves = []
    t0 = 0
    for h in range(2):
        t1 = t0 + MTH[h]
        groups = []
        for ta in range(t0, t1, SC_TILES):
            tb = min(ta + SC_TILES, t1)
            nid = 16
            for c in range(NCORES):
                last = 0
                for tt in range(ta, tb):
                    sl = tile_slot[tt]
                    first_t = int(tile_off[h, sl])
                    real = min(max(int(fills[c][h, sl]) - (tt - first_t) * P, 0), P)
                    if real > 0:
                        last = max(last, (tt - ta) * P + real)
                nid = max(nid, (last + 15) // 16 * 16)
            groups.append((ta, tb, nid))
        halves.append(dict(tile_lo=t0, tile_hi=t1, row_lo=t0 * P, row_hi=t1 * P,
                           groups=groups))
        t0 = t1

    cores = []
    for c in range(NCORES):
        cm = np.nonzero(owner == c)[0]
        idxA = np.zeros(NMSG, dtype=np.int64)
        idxA[msg_row[cm]] = src[cm]
        # layer-1 source positions in the split-AllGather h_full layout
        t = idxA
        lower = (t % BLK) < (BLK // 2)
        idxA2 = np.where(
            lower,
            (t // BLK) * (BLK // 2) + (t % BLK),
            N // 2 + (t // BLK) * (BLK // 2) + (t % BLK) - (BLK // 2),
        )

        # scatter slots in GEMM-row order (within-half); pads -> trash rows
        idxS = np.zeros(NMSG, dtype=np.int64)
        for h in range(2):
            lo, hi = halves[h]["row_lo"], halves[h]["row_hi"]
            idxS[lo:hi] = SENDH + np.arange(hi - lo)      # default: trash
        idxS[msg_row[cm]] = send_slot[cm]

        # one-hot matrices against the strided recv-load layout (unchanged)
        S = np.zeros((NSUB, J2M + J2O, P, P), dtype=np.float32)
        dm = np.nonzero(peer == c)[0]
        for m in dm:
            k = sub[m]
            d = (dest[m] - c * BLK) % P
            if of_pos[m] < 0:
                pos = send_slot[m] - c * SEG - ksub[m] * R1M
                rr = owner[m] * R1M + pos
                S[k, rr % J2M, rr // J2M, d] = 1.0
            else:
                rr2 = owner[m] * OVER + of_pos[m]
                S[k, J2M + rr2 % J2O, rr2 // J2O, d] = 1.0

        CT = np.zeros((1 + TWO_R, BLK), dtype=np.float32)
        CT[0, :] = 1.0
        for m in dm:
            CT[1 + r2[m], dest[m] - c * BLK] += 1.0

        cores.append(
            dict(
                idxA=_pack_idx16(idxA),
                idxA2=_pack_idx16(idxA2),
                idxS=_pack_idx16(idxS),
                S=S.reshape(NSUB * (J2M + J2O) * P, P).astype(BF16),
                CT=CT.astype(BF16),
            )
        )

    return dict(
        MT=MT, MTH=MTH, tile_slot=tile_slot, NMSG=NMSG, R1M=R1M, OVER=OVER,
        SEG=SEG, SENDH=SENDH, J2M=J2M, J2O=J2O, NCHUNK=NCHUNK, halves=halves,
        cores=cores,
    )


def _build(MT, MTH, tile_slot, NMSG, R1M, OVER, SEG, SENDH, J2M, J2O, NCHUNK,
           halves):
    nc = bacc.Bacc(
        "TRN2",
        target_bir_lowering=False,
        debug=False,
        enable_asserts=True,
        num_devices=NCORES,
    )
    dt = mybir.dt

    x0 = nc.dram_tensor("x0", [N, D], dt.bfloat16, kind="ExternalInput")
    x_own = nc.dram_tensor("x_own", [BLK, D], dt.bfloat16, kind="ExternalInput")
    wrel = nc.dram_tensor("wrel", [L, RPC, D, D], dt.bfloat16, kind="ExternalInput")
    wselfT = nc.dram_tensor("wselfT", [L, D, D], dt.bfloat16, kind="ExternalInput")
    bias = nc.dram_tensor("bias", [L, 1 + TWO_R, D], dt.bfloat16, kind="ExternalInput")
    ct = nc.dram_tensor("ct", [1 + TWO_R, BLK], dt.bfloat16, kind="ExternalInput")
    wffT = nc.dram_tensor("wffT", [D, OUT], dt.bfloat16, kind="ExternalInput")
    bff = nc.dram_tensor("bff", [1, OUT], dt.bfloat16, kind="ExternalInput")
    ident = nc.dram_tensor("ident", [P, P], dt.bfloat16, kind="ExternalInput")
    idxA = nc.dram_tensor("idxA", [P, NMSG // 16], dt.int16, kind="ExternalInput")
    idxA2 = nc.dram_tensor("idxA2", [P, NMSG // 16], dt.int16, kind="ExternalInput")
    idxS = nc.dram_tensor("idxS", [P, NMSG // 16], dt.int16, kind="ExternalInput")
    s_in = nc.dram_tensor("s", [NCHUNK * P, P], dt.bfloat16, kind="ExternalInput")
    y = nc.dram_tensor("y", [BLK, OUT], dt.float32, kind="ExternalOutput")

    h_own = [
        nc.dram_tensor(f"h_own{h}", [BLK // 2, D], dt.bfloat16) for h in range(2)
    ]
    h_full = nc.dram_tensor("h_full", [N, D], dt.bfloat16, addr_space="Shared")
    send = [
        [
            nc.dram_tensor(f"send{ll}_{h}", [SENDH + MTH[h] * P, D], dt.bfloat16)
            for h in range(2)
        ]
        for ll in range(L)
    ]
    recv = [
        [nc.dram_tensor(f"recv{ll}_{h}", [SENDH, D], dt.bfloat16) for h in range(2)]
        for ll in range(L)
    ]

    Relu = mybir.ActivationFunctionType.Relu

    with tile.TileContext(nc) as tc:
        with (
            tc.tile_pool(name="const", bufs=1) as const,
            tc.tile_pool(name="xtc", bufs=2) as xtcp,
            tc.tile_pool(name="xself", bufs=1) as xsp,
            tc.tile_pool(name="mso", bufs=2) as msop,
            tc.tile_pool(name="msgb", bufs=8) as msgbp,
            tc.tile_pool(name="selfb", bufs=8) as selfbp,
            tc.tile_pool(name="selfbT", bufs=8) as selfbTp,
            tc.tile_pool(name="hT", bufs=8) as hTp,
            tc.tile_pool(name="h", bufs=3) as hp,
            tc.tile_pool(name="psum_m", bufs=3, space="PSUM") as psum_m,
            tc.tile_pool(name="psum_o", bufs=2, space="PSUM") as psum_o,
            tc.tile_pool(name="psum_y", bufs=1, space="PSUM") as psum_y,
            tc.tile_pool(name="psum_tr", bufs=1, space="PSUM") as psum_tr,
        ):
            # ---- constants; startup-critical first.  Layer-1-only loads
            # are deferred so the HBM-bound startup stays small. ----
            idxA_sb = const.tile([P, NMSG // 16], dt.int16)
            nc.sync.dma_start(idxA_sb[:], idxA.ap())

            zero_sb = const.tile([P, 8, D], dt.bfloat16)
            nc.vector.memset(zero_sb[:], 0.0)
            ones_sb = const.tile([1, P], dt.bfloat16)
            nc.vector.memset(ones_sb[:], 1.0)

            def zero_wire(ll, h):
                zrows = P * 8
                for lo in range(0, SENDH, zrows):
                    hi = min(lo + zrows, SENDH)
                    nc.gpsimd.dma_start(
                        send[ll][h].ap()[lo:hi, :],
                        zero_sb[:, : (hi - lo) // P, :],
                    )

            zero_wire(0, 0)
            zero_wire(0, 1)

            xself0 = xsp.tile([P, KC, BLK], dt.bfloat16, tag="xself")
            nc.sync.dma_start_transpose(xself0[:], x_own.ap())
            wselfT_sb = const.tile([P, L, KC, D], dt.bfloat16)
            nc.sync.dma_start(
                wselfT_sb[:], wselfT.ap().rearrange("l (c p) n -> p l c n", p=P)
            )
            ct_sb = const.tile([1 + TWO_R, BLK], dt.bfloat16)
            nc.sync.dma_start(ct_sb[:], ct.ap())
            bias_sb = const.tile([1 + TWO_R, L, D], dt.bfloat16)
            nc.sync.dma_start(bias_sb[:], bias.ap().rearrange("l b d -> b l d"))
            idxS_sb = const.tile([P, NMSG // 16], dt.int16)
            nc.sync.dma_start(idxS_sb[:], idxS.ap())
            ident_sb = const.tile([P, P], dt.bfloat16)
            nc.sync.dma_start(ident_sb[:], ident.ap())

            wrel_sb = [[None] * RPC for _ in range(L)]
            for ss in range(RPC):
                wt = const.tile([P, KC, D], dt.bfloat16, tag=f"wrel0_{ss}")
                nc.scalar.dma_start(
                    wt[:], wrel.ap()[0, ss].rearrange("(c p) n -> p c n", p=P)
                )
                wrel_sb[0][ss] = wt
            s_sb = const.tile([P, NCHUNK, P], dt.bfloat16)
            nc.scalar.dma_start(s_sb[:], s_in.ap().rearrange("(c p) n -> p c n", p=P))

            def late_consts():
                for ss in range(RPC):
                    wt = const.tile([P, KC, D], dt.bfloat16, tag=f"wrel1_{ss}")
                    nc.scalar.dma_start(
                        wt[:], wrel.ap()[1, ss].rearrange("(c p) n -> p c n", p=P)
                    )
                    wrel_sb[1][ss] = wt
                idxA2_sb = const.tile([P, NMSG // 16], dt.int16)
                nc.scalar.dma_start(idxA2_sb[:], idxA2.ap())
                wffT_sb = const.tile([P, KC, OUT], dt.bfloat16)
                nc.scalar.dma_start(
                    wffT_sb[:], wffT.ap().rearrange("(c p) n -> p c n", p=P)
                )
                bff_sb = const.tile([1, OUT], dt.bfloat16)
                nc.scalar.dma_start(bff_sb[:], bff.ap())
                zero_wire(1, 0)
                zero_wire(1, 1)
                return idxA2_sb, wffT_sb, bff_sb

            # tiny warm-up collectives on sync: pay ncfw cold-start early.
            warm_in = nc.dram_tensor("warm_in", [16, 64], dt.bfloat16)
            warm_out = nc.dram_tensor("warm_out", [16, 64], dt.bfloat16)
            warm_ag = nc.dram_tensor(
                "warm_ag", [128, 64], dt.bfloat16, addr_space="Shared"
            )
            nc.sync.dma_start(warm_in.ap(), zero_sb[:16, 0, :64])
            nc.gpsimd.collective_compute(
                "AllToAll",
                mybir.AluOpType.bypass,
                replica_groups=[list(range(NCORES))],
                ins=[warm_in.ap()],
                outs=[warm_out.ap()],
            )
            nc.gpsimd.collective_compute(
                "AllGather",
                mybir.AluOpType.bypass,
                replica_groups=[list(range(NCORES))],
                ins=[warm_in.ap()],
                outs=[warm_ag.ap()],
            )

            # ---- layer-0 self+bias tiles (fp32, added on DVE in accum) -----
            selfb0 = [None] * NSUB

            def selfb0_compute(ks):
                for k in ks:
                    pm = psum_m.tile([P, D], dt.float32, space="PSUM", tag="pmsg")
                    for kc in range(KC):
                        nc.tensor.matmul(
                            out=pm[:],
                            lhsT=xself0[:, kc, k * P : (k + 1) * P],
                            rhs=wselfT_sb[:, 0, kc, :],
                            start=(kc == 0),
                            stop=False,
                        )
                    nc.tensor.matmul(
                        out=pm[:],
                        lhsT=ct_sb[:, k * P : (k + 1) * P],
                        rhs=bias_sb[:, 0, :],
                        start=False,
                        stop=True,
                    )
                    sb = selfbp.tile([P, D], dt.float32, tag="selfb")
                    nc.vector.tensor_copy(sb[:], pm[:])
                    selfb0[k] = sb

            def msg_phase(layer, src_t, idx_sb):
                for h in range(2):
                    hv = halves[h]
                    row_lo, row_hi = hv["row_lo"], hv["row_hi"]
                    tile_lo, tile_hi = hv["tile_lo"], hv["tile_hi"]
                    grp_tile = None
                    groups = {ta: (tb, nid) for ta, tb, nid in hv["groups"]}
                    g0, gend, gnid = None, None, None
                    for lo in range(row_lo, row_hi, CH_IDX):
                        hi = min(lo + CH_IDX, row_hi)
                        xc = xtcp.tile([P, KC, hi - lo], dt.bfloat16, tag="xTc")
                        nc.gpsimd.dma_gather(
                            out_ap=xc[:],
                            in_ap=src_t.ap(),
                            idxs_ap=idx_sb[:, lo // 16 : hi // 16],
                            num_idxs=hi - lo,
                            num_idxs_reg=hi - lo,
                            elem_size=D,
                            transpose=True,
                        )
                        for ti in range((hi - lo) // P):
                            mt = lo // P + ti
                            if mt in groups:
                                g0, (gend, gnid) = mt, groups[mt]
                                grp_tile = msop.tile(
                                    [P, SC_TILES, D], dt.bfloat16, tag="mso"
                                )
                            gslot = mt - g0
                            ss = tile_slot[mt]
                            pm = psum_m.tile([P, D], dt.float32, space="PSUM", tag="pmsg")
                            for kc in range(KC):
                                nc.tensor.matmul(
                                    out=pm[:],
                                    lhsT=xc[:, kc, ti * P : (ti + 1) * P],
                                    rhs=wrel_sb[layer][ss][:, kc, :],
                                    start=(kc == 0),
                                    stop=(kc == KC - 1),
                                )
                            nc.vector.tensor_copy(grp_tile[:, gslot, :], pm[:])
                            if mt == gend - 1:
                                nc.gpsimd.dma_scatter_add(
                                    send[layer][h].ap(),
                                    grp_tile[:, : (gnid + P - 1) // P, :],
                                    idxS_sb[:, g0 * P // 16 : (g0 * P + gnid) // 16],
                                    gnid,
                                    gnid,
                                    D,
                                )
                    nc.gpsimd.collective_compute(
                        "AllToAll",
                        mybir.AluOpType.bypass,
                        replica_groups=[list(range(NCORES))],
                        ins=[send[layer][h].ap()[:SENDH, :]],
                        outs=[recv[layer][h].ap()],
                    )

            def load_recv(layer, hh):
                seg = recv[layer][hh].ap().rearrange("(s g) d -> s g d", s=NCORES)
                ov = msgbp.tile([P, J2O, D], dt.bfloat16, tag="msgO", bufs=2)
                nc.scalar.dma_start(
                    ov[:], seg[:, HSUB * R1M : HSUB * R1M + OVER, :]
                )
                mbs = []
                for kl in range(HSUB):
                    mb = msgbp.tile([P, J2M, D], dt.bfloat16, tag="msgB")
                    nc.scalar.dma_start(
                        mb[:], seg[:, kl * R1M : (kl + 1) * R1M, :]
                    )
                    mbs.append(mb)
                return ov, mbs

            JT = J2M + J2O
            hT = [None] * NSUB
            selfbT = [None] * NSUB

            def accum_half_l0(hh):
                ov, mbs = load_recv(0, hh)
                for kl in range(HSUB):
                    k = hh * HSUB + kl
                    mb = mbs[kl]
                    po = psum_o.tile([P, D], dt.float32, space="PSUM", tag="pout")
                    for j in range(J2M):
                        nc.tensor.matmul(
                            out=po[:],
                            lhsT=s_sb[:, k * JT + j, :],
                            rhs=mb[:, j, :],
                            start=(j == 0),
                            stop=False,
                        )
                    for j in range(J2O):
                        nc.tensor.matmul(
                            out=po[:],
                            lhsT=s_sb[:, k * JT + J2M + j, :],
                            rhs=ov[:, j, :],
                            start=False,
                            stop=(j == J2O - 1),
                        )
                    nc.vector.tensor_add(out=po[:], in0=po[:], in1=selfb0[k][:])
                    hsb = hp.tile([P, D], dt.bfloat16, tag="hsb")
                    nc.scalar.activation(hsb[:], po[:], Relu)
                    nc.scalar.dma_start(
                        h_own[hh].ap()[kl * P : (kl + 1) * P, :], hsb[:]
                    )
                    # h^T for the layer-1 self GEMMs (PE transpose)
                    ptr = psum_tr.tile([P, KC, P], dt.bfloat16, space="PSUM", tag="ptr")
                    for kc in range(KC):
                        nc.tensor.transpose(
                            ptr[:, kc, :], hsb[:, kc * P : (kc + 1) * P], ident_sb[:]
                        )
                    ht = hTp.tile([P, KC, P], dt.bfloat16, tag="hT")
                    nc.vector.tensor_copy(ht[:], ptr[:])
                    hT[k] = ht
                    # transposed layer-1 self+bias tile for sub-block k
                    ps = psum_m.tile([P, KC, P], dt.float32, space="PSUM", tag="pmsg")
                    for kc in range(KC):
                        for cc in range(KC):
                            nc.tensor.matmul(
                                out=ps[:, kc, :],
                                lhsT=wselfT_sb[:, 1, cc, kc * P : (kc + 1) * P],
                                rhs=ht[:, cc, :],
                                start=(cc == 0),
                                stop=False,
                            )
                        nc.tensor.matmul(
                            out=ps[:, kc, :],
                            lhsT=bias_sb[:, 1, kc * P : (kc + 1) * P],
                            rhs=ct_sb[:, k * P : (k + 1) * P],
                            start=False,
                            stop=True,
                        )
                    st = selfbTp.tile([P, KC, P], dt.float32, tag="selfbT")
                    nc.vector.tensor_copy(st[:], ps[:])
                    selfbT[k] = st
                # AllGather of this half right away
                nc.gpsimd.collective_compute(
                    "AllGather",
                    mybir.AluOpType.bypass,
                    replica_groups=[list(range(NCORES))],
                    ins=[h_own[hh].ap()],
                    outs=[h_full.ap()[hh * (N // 2) : (hh + 1) * (N // 2), :]],
                )

            def accum_half_l1(hh):
                ov, mbs = load_recv(1, hh)
                for kl in range(HSUB):
                    k = hh * HSUB + kl
                    mb = mbs[kl]
                    po = psum_o.tile([P, KC, P], dt.float32, space="PSUM", tag="pout")
                    for kc in range(KC):
                        for j in range(J2M):
                            nc.tensor.matmul(
                                out=po[:, kc, :],
                                lhsT=mb[:, j, kc * P : (kc + 1) * P],
                                rhs=s_sb[:, k * JT + j, :],
                                start=(j == 0),
                                stop=False,
                            )
                        for j in range(J2O):
                            nc.tensor.matmul(
                                out=po[:, kc, :],
                                lhsT=ov[:, j, kc * P : (kc + 1) * P],
                                rhs=s_sb[:, k * JT + J2M + j, :],
                                start=False,
                                stop=(j == J2O - 1),
                            )
                    nc.vector.tensor_add(out=po[:], in0=po[:], in1=selfbT[k][:])
                    h2t = hp.tile([P, KC, P], dt.bfloat16, tag="h2t")
                    nc.scalar.activation(h2t[:], po[:], Relu)
                    # fused FF GEMM straight off h2^T
                    py_ = psum_y.tile([P, OUT], dt.float32, space="PSUM", tag="py")
                    for kc in range(KC):
                        nc.tensor.matmul(
                            out=py_[:],
                            lhsT=h2t[:, kc, :],
                            rhs=wffT_sb[:, kc, :],
                            start=(kc == 0),
                            stop=False,
                        )
                    nc.tensor.matmul(
                        out=py_[:], lhsT=ones_sb[:], rhs=bff_sb[:],
                        start=False, stop=True,
                    )
                    ysb = hp.tile([P, OUT], dt.float32, tag="ysb")
                    nc.vector.tensor_copy(ysb[:], py_[:])
                    nc.scalar.dma_start(y.ap()[k * P : (k + 1) * P, :], ysb[:])

            # ================= layer 0 =================
            selfb0_compute(range(HSUB))          # first half: before msg GEMMs
            msg_phase(0, x0, idxA_sb)
            idxA2_sb, wffT_sb, bff_sb = late_consts()
            selfb0_compute(range(HSUB, NSUB))    # second half: during a2a waits
            accum_half_l0(0)
            accum_half_l0(1)

            # ================= layer 1 (fused FF) =================
            msg_phase(1, h_full, idxA2_sb)
            accum_half_l1(0)
            accum_half_l1(1)

    nc.compile()
    return nc


def _in_maps(plan, x, W_self, b_self, W_rel, b_rel, W_ff, b_ff):
    x0 = x.astype(BF16)
    wselfT = np.ascontiguousarray(W_self.transpose(0, 2, 1)).astype(BF16)
    bias = np.concatenate([b_self[:, None, :], b_rel], axis=1).astype(BF16)
    wffT = np.ascontiguousarray(W_ff.T).astype(BF16)
    bffr = b_ff.reshape(1, OUT).astype(BF16)
    wrelT_all = np.ascontiguousarray(W_rel.transpose(0, 1, 3, 2)).astype(BF16)
    identm = np.eye(P, dtype=BF16)

    in_maps = []
    for c in range(NCORES):
        t = plan["cores"][c]
        in_maps.append(
            {
                "x0": x0,
                "x_own": np.ascontiguousarray(x0[c * BLK : (c + 1) * BLK]),
                "wrel": np.ascontiguousarray(wrelT_all[:, c * RPC : (c + 1) * RPC]),
                "wselfT": wselfT,
                "bias": bias,
                "ct": t["CT"],
                "wffT": wffT,
                "bff": bffr,
                "ident": identm,
                "idxA": t["idxA"],
                "idxA2": t["idxA2"],
                "idxS": t["idxS"],
                "s": t["S"],
            }
        )
    return in_maps


def kernel(x, dep_idx, rel_idx, gov_idx, W_self, b_self, W_rel, b_rel, W_ff, b_ff):
    global LAST_EXEC_TIME_NS, LAST_RESULTS

    x = np.asarray(x)
    dep_idx = np.asarray(dep_idx)
    rel_idx = np.asarray(rel_idx)
    gov_idx = np.asarray(gov_idx)
    W_self = np.asarray(W_self)
    b_self = np.asarray(b_self)
    W_rel = np.asarray(W_rel)
    b_rel = np.asarray(b_rel)
    W_ff = np.asarray(W_ff)
    b_ff = np.asarray(b_ff)
    assert x.shape == (N, D) and W_rel.shape == (L, TWO_R, D, D)

    key = (dep_idx.tobytes(), rel_idx.tobytes(), gov_idx.tobytes())
    if key in _CACHE:
        nc, plan = _CACHE[key]
    else:
        plan = _plan(dep_idx, rel_idx, gov_idx)
        nc = _build(
            plan["MT"], plan["MTH"], plan["tile_slot"], plan["NMSG"], plan["R1M"],
            plan["OVER"], plan["SEG"], plan["SENDH"], plan["J2M"], plan["J2O"],
            plan["NCHUNK"], plan["halves"],
        )
        _CACHE.clear()
        _CACHE[key] = (nc, plan)

    in_maps = _in_maps(plan, x, W_self, b_self, W_rel, b_rel, W_ff, b_ff)
    res = run_bass_kernel_spmd(nc, in_maps, list(range(NCORES)))
    LAST_EXEC_TIME_NS = res.exec_time_ns
    LAST_RESULTS = res
    out = np.concatenate([res.results[c]["y"] for c in range(NCORES)], axis=0)
    return out.astype(np.float32)



# revision 13
# speedup vs baseline: 1.0167x; 1.0167x over previous
"""Trainium2 Bass kernel for the 2-layer dependency-relation GCN (8 cores).

Math per layer l, token i:
    out[i] = relu( W_self[l] @ x[i] + b_self[l]
                   + sum_{e: dep[e]==i} (W_rel[l, rel[e]]   @ x[gov[e]] + b_rel[l, rel[e]])
                   + sum_{e: gov[e]==i} (W_rel[l, R+rel[e]] @ x[dep[e]] + b_rel[l, R+rel[e]]) )
final:  y = h @ W_ff.T + b_ff

Relation-sharded message passing (5 of the 40 directed relations per core).
v3 schedule: the software-DGE gather/scatter descriptor generation is hoisted
off the critical path:
  * layer-0 source gathers are plain SWDGE ops on queue 0, fired immediately
    at startup (x is an input, already resident) so they complete under the
    NEFF startup barrier.
  * all scatters and the layer-1 gathers are PREPARE_ONLY preps (desc-gen
    early, during idle windows) + trigger_dma at the dependency point; four
    SWDGE queues keep the batches independent.
  * collectives ride the Sync engine (idle otherwise); outputs are Shared
    DRAM tensors; no warm-up collectives -- the first real A2A absorbs the
    ncfw cold-start inside the compute overlap.
  * layer-1 accumulation runs in direct orientation (same one-hot S matmuls
    as layer 0), then ReLU'd h2 is PE-transposed once to feed the fused FF
    GEMM.  The transposed layer-1 self+bias tiles are computed directly from
    h^T during layer-0 accumulation (5 wide matmuls per sub-block).
Numerics: bf16 matmul inputs / wire, fp32 PSUM accumulation.
"""

import numpy as np
import ml_dtypes

import concourse.bass as bass
import concourse.mybir as mybir
import concourse.tile as tile
from concourse import bacc
from concourse.bass_utils import run_bass_kernel_spmd

N = 8192
D = 512
R = 20
TWO_R = 2 * R
L = 2
OUT = 256
P = 128
NCORES = 8
RPC = TWO_R // NCORES    # 5 relations per core
BLK = N // NCORES        # 1024 tokens per core
NSUB = BLK // P          # 8 sub-blocks of 128 tokens
HSUB = NSUB // 2
KC = D // P              # 4 contraction chunks
CH_IDX = 4 * P           # idxs per transposing-gather chunk

BF16 = ml_dtypes.bfloat16

LAST_EXEC_TIME_NS = None
LAST_RESULTS = None

_CACHE = {}


def _pack_idx16(idx: np.ndarray) -> np.ndarray:
    Ln = len(idx)
    assert Ln % 16 == 0
    base = idx.astype(np.int16).reshape(Ln // 16, 16).T
    return np.tile(base, (8, 1)).copy()


def _plan(dep_idx: np.ndarray, rel_idx: np.ndarray, gov_idx: np.ndarray):
    dep = dep_idx.astype(np.int64)
    gov = gov_idx.astype(np.int64)
    rel = rel_idx.astype(np.int64)

    dest = np.concatenate([dep, gov])
    src = np.concatenate([gov, dep])
    r2 = np.concatenate([rel, rel + R])

    owner = r2 // RPC
    slot = r2 % RPC
    peer = dest // BLK
    sub = (dest % BLK) // P
    half = sub // HSUB
    ksub = sub % HSUB                 # sub index within the half

    # GEMM tiling: tiles per (dest-half, relation-slot), max over cores
    tps = np.ones((2, RPC), dtype=np.int64)
    for h in range(2):
        for s in range(RPC):
            for c in range(NCORES):
                n = int(((owner == c) & (half == h) & (slot == s)).sum())
                tps[h, s] = max(tps[h, s], (n + P - 1) // P)
    tile_slot = []
    tile_off = np.zeros((2, RPC), dtype=np.int64)
    off = 0
    for h in range(2):
        for s in range(RPC):
            tile_off[h, s] = off
            tile_slot.extend([s] * int(tps[h, s]))
            off += int(tps[h, s])
    MT = off
    MTH = [int(tps[0].sum()), int(tps[1].sum())]
    NMSG = MT * P

    # wire layout per half for peer p: [ksub 0..HSUB-1][R1M rows] ++ [OVER]
    cnt = np.zeros((NCORES, NCORES, NSUB), dtype=np.int64)
    np.add.at(cnt, (owner, peer, sub), 1)

    def over_for(r1m):
        ov = 0
        for c in range(NCORES):
            for p in range(NCORES):
                for hh in range(2):
                    tot = sum(
                        max(0, int(cnt[c, p, hh * HSUB + kl]) - r1m)
                        for kl in range(HSUB)
                    )
                    ov = max(ov, tot)
        return int(np.ceil(ov / 16) * 16) if ov else 0

    best = None
    for r1m in (16, 32, 48, 64):
        ov = over_for(r1m)
        seg = HSUB * r1m + ov
        chunks = (NCORES * r1m) // P + (NCORES * ov) // P
        key = (seg, chunks)
        if best is None or key < best[0]:
            best = (key, r1m, ov, seg)
    _, R1M, OVER, SEG = best
    if OVER == 0:
        OVER = 16
        SEG = HSUB * R1M + OVER
    SENDH = NCORES * SEG              # wire rows per half per rank
    J2M = NCORES * R1M // P
    J2O = NCORES * OVER // P
    assert (NCORES * R1M) % P == 0 and (NCORES * OVER) % P == 0
    NCHUNK = NSUB * (J2M + J2O)

    # per-message assignment; send_slot is WITHIN its half's image
    msg_row = np.zeros(2 * N, dtype=np.int64)
    send_slot = np.zeros(2 * N, dtype=np.int64)
    of_pos = np.zeros(2 * N, dtype=np.int64) - 1
    fills = []
    for c in range(NCORES):
        cm = np.nonzero(owner == c)[0]
        fill = np.zeros((2, RPC), dtype=np.int64)
        rfill = np.zeros((NCORES, NSUB), dtype=np.int64)
        ofill = np.zeros((NCORES, 2), dtype=np.int64)
        for m in cm:
            hh = half[m]
            sl = slot[m]
            msg_row[m] = tile_off[hh, sl] * P + fill[hh, sl]
            fill[hh, sl] += 1
            p = peer[m]
            pos = rfill[p, sub[m]]
            rfill[p, sub[m]] += 1
            base = p * SEG
            if pos < R1M:
                send_slot[m] = base + ksub[m] * R1M + pos
            else:
                op_ = ofill[p, hh]
                assert op_ < OVER
                ofill[p, hh] += 1
                of_pos[m] = op_
                send_slot[m] = base + HSUB * R1M + op_
        fills.append(fill)

    # half row/tile ranges
    halves = []
    t0 = 0
    for h in range(2):
        t1 = t0 + MTH[h]
        halves.append(dict(tile_lo=t0, tile_hi=t1, row_lo=t0 * P, row_hi=t1 * P))
        t0 = t1

    cores = []
    for c in range(NCORES):
        cm = np.nonzero(owner == c)[0]
        idxA = np.zeros(NMSG, dtype=np.int64)
        idxA[msg_row[cm]] = src[cm]
        # layer-1 source positions in the split-AllGather h_full layout
        t = idxA
        lower = (t % BLK) < (BLK // 2)
        idxA2 = np.where(
            lower,
            (t // BLK) * (BLK // 2) + (t % BLK),
            N // 2 + (t // BLK) * (BLK // 2) + (t % BLK) - (BLK // 2),
        )

        # scatter slots in GEMM-row order (within-half); pads -> trash rows
        idxS = np.zeros(NMSG, dtype=np.int64)
        for h in range(2):
            lo, hi = halves[h]["row_lo"], halves[h]["row_hi"]
            idxS[lo:hi] = SENDH + np.arange(hi - lo)      # default: trash
        idxS[msg_row[cm]] = send_slot[cm]

        # one-hot matrices against the strided recv-load layout
        S = np.zeros((NSUB, J2M + J2O, P, P), dtype=np.float32)
        dm = np.nonzero(peer == c)[0]
        for m in dm:
            k = sub[m]
            d = (dest[m] - c * BLK) % P
            if of_pos[m] < 0:
                pos = send_slot[m] - c * SEG - ksub[m] * R1M
                rr = owner[m] * R1M + pos
                S[k, rr % J2M, rr // J2M, d] = 1.0
            else:
                rr2 = owner[m] * OVER + of_pos[m]
                S[k, J2M + rr2 % J2O, rr2 // J2O, d] = 1.0

        CT = np.zeros((1 + TWO_R, BLK), dtype=np.float32)
        CT[0, :] = 1.0
        for m in dm:
            CT[1 + r2[m], dest[m] - c * BLK] += 1.0

        cores.append(
            dict(
                idxA=_pack_idx16(idxA),
                idxA2=_pack_idx16(idxA2),
                idxS=_pack_idx16(idxS),
                S=S.reshape(NSUB * (J2M + J2O) * P, P).astype(BF16),
                CT=CT.astype(BF16),
            )
        )

    return dict(
        MT=MT, MTH=MTH, tile_slot=tile_slot, NMSG=NMSG, R1M=R1M, OVER=OVER,
        SEG=SEG, SENDH=SENDH, J2M=J2M, J2O=J2O, NCHUNK=NCHUNK, halves=halves,
        cores=cores,
    )


def _build(MT, MTH, tile_slot, NMSG, R1M, OVER, SEG, SENDH, J2M, J2O, NCHUNK,
           halves):
    nc = bacc.Bacc(
        "TRN2",
        target_bir_lowering=False,
        debug=False,
        enable_asserts=True,
        num_devices=NCORES,
        num_swdge_queues=4,
    )
    dt = mybir.dt

    x0 = nc.dram_tensor("x0", [N, D], dt.bfloat16, kind="ExternalInput")
    x_own = nc.dram_tensor("x_own", [BLK, D], dt.bfloat16, kind="ExternalInput")
    wrel = nc.dram_tensor("wrel", [L, RPC, D, D], dt.bfloat16, kind="ExternalInput")
    wselfT = nc.dram_tensor("wselfT", [L, D, D], dt.bfloat16, kind="ExternalInput")
    bias = nc.dram_tensor("bias", [L, 1 + TWO_R, D], dt.bfloat16, kind="ExternalInput")
    ct = nc.dram_tensor("ct", [1 + TWO_R, BLK], dt.bfloat16, kind="ExternalInput")
    wffT = nc.dram_tensor("wffT", [D, OUT], dt.bfloat16, kind="ExternalInput")
    bff = nc.dram_tensor("bff", [1, OUT], dt.bfloat16, kind="ExternalInput")
    ident = nc.dram_tensor("ident", [P, P], dt.bfloat16, kind="ExternalInput")
    idxA = nc.dram_tensor("idxA", [P, NMSG // 16], dt.int16, kind="ExternalInput")
    idxA2 = nc.dram_tensor("idxA2", [P, NMSG // 16], dt.int16, kind="ExternalInput")
    idxS = nc.dram_tensor("idxS", [P, NMSG // 16], dt.int16, kind="ExternalInput")
    s_in = nc.dram_tensor("s", [NCHUNK * P, P], dt.bfloat16, kind="ExternalInput")
    y = nc.dram_tensor("y", [BLK, OUT], dt.float32, kind="ExternalOutput")

    h_own = [
        nc.dram_tensor(f"h_own{h}", [BLK // 2, D], dt.bfloat16) for h in range(2)
    ]
    h_full = nc.dram_tensor("h_full", [N, D], dt.bfloat16, addr_space="Shared")
    send = [
        [
            nc.dram_tensor(f"send{ll}_{h}", [SENDH + MTH[h] * P, D], dt.bfloat16)
            for h in range(2)
        ]
        for ll in range(L)
    ]
    recv = [
        [nc.dram_tensor(f"recv{ll}_{h}", [SENDH, D], dt.bfloat16) for h in range(2)]
        for ll in range(L)
    ]

    Relu = mybir.ActivationFunctionType.Relu
    RG = [list(range(NCORES))]

    # SWDGE queue assignment
    Q_G0 = 0      # layer-0 gathers (plain, fired at startup)
    Q_S0 = 1      # layer-0 scatters (prep+trigger, batch per half)
    Q_G1 = 2      # layer-1 gathers (prep early, trigger on h_full)
    Q_S1 = 3      # layer-1 scatters (prep+trigger, batch per half)

    def half_chunks(h):
        lo, hi = halves[h]["row_lo"], halves[h]["row_hi"]
        out = []
        ci = 0
        for clo in range(lo, hi, CH_IDX):
            out.append((ci, clo, min(clo + CH_IDX, hi)))
            ci += 1
        return out

    with tile.TileContext(nc) as tc:
        with (
            tc.tile_pool(name="const", bufs=1) as const,
            tc.tile_pool(name="xtc", bufs=1) as xtcp,
            tc.tile_pool(name="xself", bufs=1) as xsp,
            tc.tile_pool(name="mso", bufs=1) as msop,
            tc.tile_pool(name="msgb", bufs=4) as msgbp,
            tc.tile_pool(name="selfb", bufs=8) as selfbp,
            tc.tile_pool(name="selfb1", bufs=8) as selfb1p,
            tc.tile_pool(name="hT", bufs=2) as hTp,
            tc.tile_pool(name="h", bufs=3) as hp,
            tc.tile_pool(name="psum_m", bufs=3, space="PSUM") as psum_m,
            tc.tile_pool(name="psum_o", bufs=2, space="PSUM") as psum_o,
            tc.tile_pool(name="psum_y", bufs=1, space="PSUM") as psum_y,
            tc.tile_pool(name="psum_tr", bufs=1, space="PSUM") as psum_tr,
        ):
            # ---- startup loads: small index tensors first ----
            idxA_sb = const.tile([P, NMSG // 16], dt.int16)
            nc.sync.dma_start(idxA_sb[:], idxA.ap())
            idxS_sb = const.tile([P, NMSG // 16], dt.int16)
            nc.sync.dma_start(idxS_sb[:], idxS.ap())
            idxA2_sb = const.tile([P, NMSG // 16], dt.int16)
            nc.scalar.dma_start(idxA2_sb[:], idxA2.ap())

            zero_sb = const.tile([P, 4, D], dt.bfloat16)
            nc.vector.memset(zero_sb[:], 0.0)
            ones_sb = const.tile([1, P], dt.bfloat16)
            nc.vector.memset(ones_sb[:], 1.0)

            # tiny warm-up collectives: pay ncfw cold-start during startup.
            warm_in = nc.dram_tensor("warm_in", [16, 64], dt.bfloat16)
            warm_out = nc.dram_tensor("warm_out", [16, 64], dt.bfloat16)
            warm_ag = nc.dram_tensor(
                "warm_ag", [128, 64], dt.bfloat16, addr_space="Shared"
            )
            nc.sync.dma_start(warm_in.ap(), zero_sb[:16, 0, :64])
            nc.gpsimd.collective_compute(
                "AllToAll",
                mybir.AluOpType.bypass,
                replica_groups=RG,
                ins=[warm_in.ap()],
                outs=[warm_out.ap()],
            )
            nc.gpsimd.collective_compute(
                "AllGather",
                mybir.AluOpType.bypass,
                replica_groups=RG,
                ins=[warm_in.ap()],
                outs=[warm_ag.ap()],
            )

            def zero_wire(ll, h):
                zrows = P * 4
                for lo in range(0, SENDH, zrows):
                    hi = min(lo + zrows, SENDH)
                    nc.sync.dma_start(
                        send[ll][h].ap()[lo:hi, :],
                        zero_sb[:, : (hi - lo) // P, :],
                    )

            xself0 = xsp.tile([P, KC, BLK], dt.bfloat16, tag="xself")
            nc.sync.dma_start_transpose(xself0[:], x_own.ap())
            wselfT_sb = const.tile([P, L, KC, D], dt.bfloat16)
            nc.sync.dma_start(
                wselfT_sb[:], wselfT.ap().rearrange("l (c p) n -> p l c n", p=P)
            )
            ct_sb = const.tile([1 + TWO_R, BLK], dt.bfloat16)
            nc.sync.dma_start(ct_sb[:], ct.ap())
            bias_sb = const.tile([1 + TWO_R, L, D], dt.bfloat16)
            nc.sync.dma_start(bias_sb[:], bias.ap().rearrange("l b d -> b l d"))
            ident_sb = const.tile([P, P], dt.bfloat16)
            nc.sync.dma_start(ident_sb[:], ident.ap())
            s_sb = const.tile([P, NCHUNK, P], dt.bfloat16)
            nc.sync.dma_start(s_sb[:], s_in.ap().rearrange("(c p) n -> p c n", p=P))
            zero_wire(0, 0)
            zero_wire(0, 1)

            wrel_sb = [[None] * RPC for _ in range(L)]
            for ss in range(RPC):
                wt = const.tile([P, KC, D], dt.bfloat16, tag=f"wrel0_{ss}")
                nc.scalar.dma_start(
                    wt[:], wrel.ap()[0, ss].rearrange("(c p) n -> p c n", p=P)
                )
                wrel_sb[0][ss] = wt

            # ---- layer-0 gathers: plain SWDGE on q0, data (x0) is resident ----
            def make_xc(h, ci, nrow):
                xc = xtcp.tile(
                    [P, KC, nrow], dt.bfloat16, tag=f"xc{h}_{ci}", bufs=1,
                    name=f"xc{h}_{ci}",
                )
                return xc

            xc0 = {}
            for h in range(2):
                for ci, clo, chi in half_chunks(h):
                    xc = make_xc(h, ci, chi - clo)
                    nc.gpsimd.dma_gather(
                        out_ap=xc[:],
                        in_ap=x0.ap(),
                        idxs_ap=idxA_sb[:, clo // 16 : chi // 16],
                        num_idxs=chi - clo,
                        num_idxs_reg=chi - clo,
                        elem_size=D,
                        transpose=True,
                        queue_num=Q_G0,
                    )
                    xc0[(h, ci)] = xc

            # ---- mso output tiles + layer-0 h0 scatter prep (q1) ----
            mso = [
                msop.tile(
                    [P, MTH[h], D], dt.bfloat16, tag=f"mso{h}", bufs=1,
                    name=f"mso{h}",
                )
                for h in range(2)
            ]

            def scatter(layer, h, q):
                lo, hi = halves[h]["row_lo"], halves[h]["row_hi"]
                nc.gpsimd.dma_scatter_add(
                    send[layer][h].ap(),
                    mso[h][:],
                    idxS_sb[:, lo // 16 : hi // 16],
                    hi - lo,
                    hi - lo,
                    D,
                    queue_num=q,
                )

            # ---- layer-0 self+bias tiles (fp32, added on DVE in accum) -----
            selfb0 = [None] * NSUB
            selfb1 = [None] * NSUB
            hT = [None] * NSUB

            def selfb0_compute(ks):
                for k in ks:
                    pm = psum_m.tile([P, D], dt.float32, space="PSUM", tag="pmsg")
                    for kc in range(KC):
                        nc.tensor.matmul(
                            out=pm[:],
                            lhsT=xself0[:, kc, k * P : (k + 1) * P],
                            rhs=wselfT_sb[:, 0, kc, :],
                            start=(kc == 0),
                            stop=False,
                        )
                    nc.tensor.matmul(
                        out=pm[:],
                        lhsT=ct_sb[:, k * P : (k + 1) * P],
                        rhs=bias_sb[:, 0, :],
                        start=False,
                        stop=True,
                    )
                    sb = selfbp.tile([P, D], dt.float32, tag="selfb")
                    nc.vector.tensor_copy(sb[:], pm[:])
                    selfb0[k] = sb

            def msg_gemms(layer, h, xcs):
                t0, t1 = halves[h]["tile_lo"], halves[h]["tile_hi"]
                for mt in range(t0, t1):
                    tih = mt - t0
                    ci, off = (tih * P) // CH_IDX, (tih * P) % CH_IDX
                    xc = xcs[(h, ci)]
                    ss = tile_slot[mt]
                    pm = psum_m.tile([P, D], dt.float32, space="PSUM", tag="pmsg")
                    for kc in range(KC):
                        nc.tensor.matmul(
                            out=pm[:],
                            lhsT=xc[:, kc, off : off + P],
                            rhs=wrel_sb[layer][ss][:, kc, :],
                            start=(kc == 0),
                            stop=(kc == KC - 1),
                        )
                    nc.vector.tensor_copy(mso[h][:, tih, :], pm[:])

            def a2a(layer, h):
                nc.gpsimd.collective_compute(
                    "AllToAll",
                    mybir.AluOpType.bypass,
                    replica_groups=RG,
                    ins=[send[layer][h].ap()[:SENDH, :]],
                    outs=[recv[layer][h].ap()],
                )

            # ================= layer 0 message phase =================
            selfb0_compute(range(NSUB))
            msg_gemms(0, 0, xc0)
            scatter(0, 0, Q_S0)
            a2a(0, 0)
            msg_gemms(0, 1, xc0)
            scatter(0, 1, Q_S0)
            a2a(0, 1)

            # layer-1-only consts (vector queue) + wire zeroing.  wrel layer-1
            # reuses the layer-0 weight buffers (WAR: load waits l0 GEMMs).
            for ss in range(RPC):
                wt = const.tile([P, KC, D], dt.bfloat16, tag=f"wrel0_{ss}")
                nc.scalar.dma_start(
                    wt[:], wrel.ap()[1, ss].rearrange("(c p) n -> p c n", p=P)
                )
                wrel_sb[1][ss] = wt
            wffT_sb = const.tile([P, KC, OUT], dt.bfloat16)
            nc.scalar.dma_start(
                wffT_sb[:], wffT.ap().rearrange("(c p) n -> p c n", p=P)
            )
            bff_sb = const.tile([1, OUT], dt.bfloat16)
            nc.scalar.dma_start(bff_sb[:], bff.ap())
            zero_wire(1, 0)
            zero_wire(1, 1)


            def load_recv(layer, hh):
                seg = recv[layer][hh].ap().rearrange("(s g) d -> s g d", s=NCORES)
                ov = msgbp.tile([P, J2O, D], dt.bfloat16, tag="msgO", bufs=2)
                nc.scalar.dma_start(
                    ov[:], seg[:, HSUB * R1M : HSUB * R1M + OVER, :]
                )
                mbs = []
                for kl in range(HSUB):
                    mb = msgbp.tile([P, J2M, D], dt.bfloat16, tag="msgB")
                    nc.scalar.dma_start(
                        mb[:], seg[:, kl * R1M : (kl + 1) * R1M, :]
                    )
                    mbs.append(mb)
                return ov, mbs

            JT = J2M + J2O

            def accum_half_l0(hh):
                ov, mbs = load_recv(0, hh)
                for kl in range(HSUB):
                    k = hh * HSUB + kl
                    mb = mbs[kl]
                    po = psum_o.tile([P, D], dt.float32, space="PSUM", tag="pout")
                    for j in range(J2M):
                        nc.tensor.matmul(
                            out=po[:],
                            lhsT=s_sb[:, k * JT + j, :],
                            rhs=mb[:, j, :],
                            start=(j == 0),
                            stop=False,
                        )
                    for j in range(J2O):
                        nc.tensor.matmul(
                            out=po[:],
                            lhsT=s_sb[:, k * JT + J2M + j, :],
                            rhs=ov[:, j, :],
                            start=False,
                            stop=(j == J2O - 1),
                        )
                    nc.vector.tensor_add(out=po[:], in0=po[:], in1=selfb0[k][:])
                    hsb = hp.tile([P, D], dt.bfloat16, tag="hsb")
                    nc.scalar.activation(hsb[:], po[:], Relu)
                    nc.scalar.dma_start(
                        h_own[hh].ap()[kl * P : (kl + 1) * P, :], hsb[:]
                    )
                    # h^T (PE transpose) for the direct layer-1 self GEMM
                    ptr = psum_tr.tile([P, KC, P], dt.bfloat16, space="PSUM", tag="ptr")
                    for kc in range(KC):
                        nc.tensor.transpose(
                            ptr[:, kc, :], hsb[:, kc * P : (kc + 1) * P], ident_sb[:]
                        )
                    ht = hTp.tile([P, KC, P], dt.bfloat16, tag="hT")
                    nc.vector.tensor_copy(ht[:], ptr[:])
                    hT[k] = ht
                    # layer-1 self+bias tile for sub-block k (direct layout)
                    ps = psum_m.tile([P, D], dt.float32, space="PSUM", tag="pmsg")
                    for cc in range(KC):
                        nc.tensor.matmul(
                            out=ps[:],
                            lhsT=ht[:, cc, :],
                            rhs=wselfT_sb[:, 1, cc, :],
                            start=(cc == 0),
                            stop=False,
                        )
                    nc.tensor.matmul(
                        out=ps[:],
                        lhsT=ct_sb[:, k * P : (k + 1) * P],
                        rhs=bias_sb[:, 1, :],
                        start=False,
                        stop=True,
                    )
                    sb1 = selfb1p.tile([P, D], dt.float32, tag="selfb1")
                    nc.vector.tensor_copy(sb1[:], ps[:])
                    selfb1[k] = sb1
                # AllGather of this half right away
                nc.gpsimd.collective_compute(
                    "AllGather",
                    mybir.AluOpType.bypass,
                    replica_groups=RG,
                    ins=[h_own[hh].ap()],
                    outs=[h_full.ap()[hh * (N // 2) : (hh + 1) * (N // 2), :]],
                )

            def accum_half_l1(hh):
                ov, mbs = load_recv(1, hh)
                for kl in range(HSUB):
                    k = hh * HSUB + kl
                    mb = mbs[kl]
                    po = psum_o.tile([P, D], dt.float32, space="PSUM", tag="pout")
                    for j in range(J2M):
                        nc.tensor.matmul(
                            out=po[:],
                            lhsT=s_sb[:, k * JT + j, :],
                            rhs=mb[:, j, :],
                            start=(j == 0),
                            stop=False,
                        )
                    for j in range(J2O):
                        nc.tensor.matmul(
                            out=po[:],
                            lhsT=s_sb[:, k * JT + J2M + j, :],
                            rhs=ov[:, j, :],
                            start=False,
                            stop=(j == J2O - 1),
                        )
                    nc.vector.tensor_add(out=po[:], in0=po[:], in1=selfb1[k][:])
                    h2 = hp.tile([P, D], dt.bfloat16, tag="h2")
                    nc.scalar.activation(h2[:], po[:], Relu)
                    ptr = psum_tr.tile([P, KC, P], dt.bfloat16, space="PSUM", tag="ptr")
                    for kc in range(KC):
                        nc.tensor.transpose(
                            ptr[:, kc, :], h2[:, kc * P : (kc + 1) * P], ident_sb[:]
                        )
                    h2t = hp.tile([P, KC, P], dt.bfloat16, tag="h2t")
                    nc.vector.tensor_copy(h2t[:], ptr[:])
                    # fused FF GEMM off h2^T
                    py_ = psum_y.tile([P, OUT], dt.float32, space="PSUM", tag="py")
                    for kc in range(KC):
                        nc.tensor.matmul(
                            out=py_[:],
                            lhsT=h2t[:, kc, :],
                            rhs=wffT_sb[:, kc, :],
                            start=(kc == 0),
                            stop=False,
                        )
                    nc.tensor.matmul(
                        out=py_[:], lhsT=ones_sb[:], rhs=bff_sb[:],
                        start=False, stop=True,
                    )
                    ysb = hp.tile([P, OUT], dt.float32, tag="ysb")
                    nc.vector.tensor_copy(ysb[:], py_[:])
                    nc.scalar.dma_start(y.ap()[k * P : (k + 1) * P, :], ysb[:])

            # ================= layer 0 accumulation =================
            accum_half_l0(0)
            accum_half_l0(1)

            # ================= layer 1 =================
            # plain gathers from h_full (wait the AllGathers), queues 2/0
            xc1 = {}
            qi = 0
            for h in range(2):
                for ci, clo, chi in half_chunks(h):
                    xc = make_xc(h, ci, chi - clo)
                    nc.gpsimd.dma_gather(
                        out_ap=xc[:],
                        in_ap=h_full.ap(),
                        idxs_ap=idxA2_sb[:, clo // 16 : chi // 16],
                        num_idxs=chi - clo,
                        num_idxs_reg=chi - clo,
                        elem_size=D,
                        transpose=True,
                        queue_num=(Q_G1 if qi % 2 == 0 else Q_G0),
                    )
                    qi += 1
                    xc1[(h, ci)] = xc
            msg_gemms(1, 0, xc1)
            scatter(1, 0, Q_S1)
            a2a(1, 0)
            msg_gemms(1, 1, xc1)
            scatter(1, 1, Q_S1)
            a2a(1, 1)

            accum_half_l1(0)
            accum_half_l1(1)

    nc.compile()
    return nc


def _in_maps(plan, x, W_self, b_self, W_rel, b_rel, W_ff, b_ff):
    x0 = x.astype(BF16)
    wselfT = np.ascontiguousarray(W_self.transpose(0, 2, 1)).astype(BF16)
    bias = np.concatenate([b_self[:, None, :], b_rel], axis=1).astype(BF16)
    wffT = np.ascontiguousarray(W_ff.T).astype(BF16)
    bffr = b_ff.reshape(1, OUT).astype(BF16)
    wrelT_all = np.ascontiguousarray(W_rel.transpose(0, 1, 3, 2)).astype(BF16)
    identm = np.eye(P, dtype=BF16)

    in_maps = []
    for c in range(NCORES):
        t = plan["cores"][c]
        in_maps.append(
            {
                "x0": x0,
                "x_own": np.ascontiguousarray(x0[c * BLK : (c + 1) * BLK]),
                "wrel": np.ascontiguousarray(wrelT_all[:, c * RPC : (c + 1) * RPC]),
                "wselfT": wselfT,
                "bias": bias,
                "ct": t["CT"],
                "wffT": wffT,
                "bff": bffr,
                "ident": identm,
                "idxA": t["idxA"],
                "idxA2": t["idxA2"],
                "idxS": t["idxS"],
                "s": t["S"],
            }
        )
    return in_maps


def kernel(x, dep_idx, rel_idx, gov_idx, W_self, b_self, W_rel, b_rel, W_ff, b_ff):
    global LAST_EXEC_TIME_NS, LAST_RESULTS

    x = np.asarray(x)
    dep_idx = np.asarray(dep_idx)
    rel_idx = np.asarray(rel_idx)
    gov_idx = np.asarray(gov_idx)
    W_self = np.asarray(W_self)
    b_self = np.asarray(b_self)
    W_rel = np.asarray(W_rel)
    b_rel = np.asarray(b_rel)
    W_ff = np.asarray(W_ff)
    b_ff = np.asarray(b_ff)
    assert x.shape == (N, D) and W_rel.shape == (L, TWO_R, D, D)

    key = (dep_idx.tobytes(), rel_idx.tobytes(), gov_idx.tobytes())
    if key in _CACHE:
        nc, plan = _CACHE[key]
    else:
        plan = _plan(dep_idx, rel_idx, gov_idx)
        nc = _build(
            plan["MT"], plan["MTH"], plan["tile_slot"], plan["NMSG"], plan["R1M"],
            plan["OVER"], plan["SEG"], plan["SENDH"], plan["J2M"], plan["J2O"],
            plan["NCHUNK"], plan["halves"],
        )
        _CACHE.clear()
        _CACHE[key] = (nc, plan)

    in_maps = _in_maps(plan, x, W_self, b_self, W_rel, b_rel, W_ff, b_ff)
    res = run_bass_kernel_spmd(nc, in_maps, list(range(NCORES)))
    LAST_EXEC_TIME_NS = res.exec_time_ns
    LAST_RESULTS = res
    out = np.concatenate([res.results[c]["y"] for c in range(NCORES)], axis=0)
    return out.astype(np.float32)


# revision 17
# speedup vs baseline: 1.0398x; 1.0227x over previous
"""Trainium2 Bass kernel for the 2-layer dependency-relation GCN (8 cores).

Math per layer l, token i:
    out[i] = relu( W_self[l] @ x[i] + b_self[l]
                   + sum_{e: dep[e]==i} (W_rel[l, rel[e]]   @ x[gov[e]] + b_rel[l, rel[e]])
                   + sum_{e: gov[e]==i} (W_rel[l, R+rel[e]] @ x[dep[e]] + b_rel[l, R+rel[e]]) )
final:  y = h @ W_ff.T + b_ff

Relation-sharded message passing (5 of the 40 directed relations per core).
v3 schedule: the software-DGE gather/scatter descriptor generation is hoisted
off the critical path:
  * layer-0 source gathers are plain SWDGE ops on queue 0, fired immediately
    at startup (x is an input, already resident) so they complete under the
    NEFF startup barrier.
  * all scatters and the layer-1 gathers are PREPARE_ONLY preps (desc-gen
    early, during idle windows) + trigger_dma at the dependency point; four
    SWDGE queues keep the batches independent.
  * collectives ride the Sync engine (idle otherwise); outputs are Shared
    DRAM tensors; no warm-up collectives -- the first real A2A absorbs the
    ncfw cold-start inside the compute overlap.
  * layer-1 accumulation runs in direct orientation (same one-hot S matmuls
    as layer 0), then ReLU'd h2 is PE-transposed once to feed the fused FF
    GEMM.  The transposed layer-1 self+bias tiles are computed directly from
    h^T during layer-0 accumulation (5 wide matmuls per sub-block).
Numerics: bf16 matmul inputs / wire, fp32 PSUM accumulation.
"""

import numpy as np
import ml_dtypes

import concourse.bass as bass
import concourse.mybir as mybir
import concourse.tile as tile
from concourse import bacc
from concourse.bass_utils import run_bass_kernel_spmd

N = 8192
D = 512
R = 20
TWO_R = 2 * R
L = 2
OUT = 256
P = 128
NCORES = 8
RPC = TWO_R // NCORES    # 5 relations per core
BLK = N // NCORES        # 1024 tokens per core
NSUB = BLK // P          # 8 sub-blocks of 128 tokens
HSUB = NSUB // 2
KC = D // P              # 4 contraction chunks
CH_IDX = 4 * P           # idxs per transposing-gather chunk

BF16 = ml_dtypes.bfloat16

LAST_EXEC_TIME_NS = None
LAST_RESULTS = None

_CACHE = {}


def _pack_idx16(idx: np.ndarray) -> np.ndarray:
    Ln = len(idx)
    assert Ln % 16 == 0
    base = idx.astype(np.int16).reshape(Ln // 16, 16).T
    return np.tile(base, (8, 1)).copy()


def _plan(dep_idx: np.ndarray, rel_idx: np.ndarray, gov_idx: np.ndarray):
    dep = dep_idx.astype(np.int64)
    gov = gov_idx.astype(np.int64)
    rel = rel_idx.astype(np.int64)

    dest = np.concatenate([dep, gov])
    src = np.concatenate([gov, dep])
    r2 = np.concatenate([rel, rel + R])

    owner = r2 // RPC
    slot = r2 % RPC
    peer = dest // BLK
    sub = (dest % BLK) // P
    half = sub // HSUB
    ksub = sub % HSUB                 # sub index within the half

    # GEMM tiling: tiles per (dest-half, relation-slot), max over cores
    tps = np.ones((2, RPC), dtype=np.int64)
    for h in range(2):
        for s in range(RPC):
            for c in range(NCORES):
                n = int(((owner == c) & (half == h) & (slot == s)).sum())
                tps[h, s] = max(tps[h, s], (n + P - 1) // P)
    tile_slot = []
    tile_off = np.zeros((2, RPC), dtype=np.int64)
    off = 0
    for h in range(2):
        for s in range(RPC):
            tile_off[h, s] = off
            tile_slot.extend([s] * int(tps[h, s]))
            off += int(tps[h, s])
    MT = off
    MTH = [int(tps[0].sum()), int(tps[1].sum())]
    NMSG = MT * P

    # wire layout per half for peer p: [ksub 0..HSUB-1][R1M rows] ++ [OVER]
    cnt = np.zeros((NCORES, NCORES, NSUB), dtype=np.int64)
    np.add.at(cnt, (owner, peer, sub), 1)

    def over_for(r1m):
        ov = 0
        for c in range(NCORES):
            for p in range(NCORES):
                for hh in range(2):
                    tot = sum(
                        max(0, int(cnt[c, p, hh * HSUB + kl]) - r1m)
                        for kl in range(HSUB)
                    )
                    ov = max(ov, tot)
        return int(np.ceil(ov / 16) * 16) if ov else 0

    best = None
    for r1m in (16, 32, 48, 64):
        ov = over_for(r1m)
        seg = HSUB * r1m + ov
        chunks = (NCORES * r1m) // P + (NCORES * ov) // P
        key = (seg, chunks)
        if best is None or key < best[0]:
            best = (key, r1m, ov, seg)
    _, R1M, OVER, SEG = best
    if OVER == 0:
        OVER = 16
        SEG = HSUB * R1M + OVER
    SENDH = NCORES * SEG              # wire rows per half per rank
    J2M = NCORES * R1M // P
    J2O = NCORES * OVER // P
    assert (NCORES * R1M) % P == 0 and (NCORES * OVER) % P == 0
    NCHUNK = NSUB * (J2M + J2O)

    # per-message assignment; send_slot is WITHIN its half's image
    msg_row = np.zeros(2 * N, dtype=np.int64)
    send_slot = np.zeros(2 * N, dtype=np.int64)
    of_pos = np.zeros(2 * N, dtype=np.int64) - 1
    fills = []
    for c in range(NCORES):
        cm = np.nonzero(owner == c)[0]
        fill = np.zeros((2, RPC), dtype=np.int64)
        rfill = np.zeros((NCORES, NSUB), dtype=np.int64)
        ofill = np.zeros((NCORES, 2), dtype=np.int64)
        for m in cm:
            hh = half[m]
            sl = slot[m]
            msg_row[m] = tile_off[hh, sl] * P + fill[hh, sl]
            fill[hh, sl] += 1
            p = peer[m]
            pos = rfill[p, sub[m]]
            rfill[p, sub[m]] += 1
            base = p * SEG
            if pos < R1M:
                send_slot[m] = base + ksub[m] * R1M + pos
            else:
                op_ = ofill[p, hh]
                assert op_ < OVER
                ofill[p, hh] += 1
                of_pos[m] = op_
                send_slot[m] = base + HSUB * R1M + op_
        fills.append(fill)

    # half row/tile ranges
    halves = []
    t0 = 0
    for h in range(2):
        t1 = t0 + MTH[h]
        halves.append(dict(tile_lo=t0, tile_hi=t1, row_lo=t0 * P, row_hi=t1 * P))
        t0 = t1

    cores = []
    for c in range(NCORES):
        cm = np.nonzero(owner == c)[0]
        idxA = np.zeros(NMSG, dtype=np.int64)
        idxA[msg_row[cm]] = src[cm]
        # layer-1 source positions in the split-AllGather h_full layout
        t = idxA
        lower = (t % BLK) < (BLK // 2)
        idxA2 = np.where(
            lower,
            (t // BLK) * (BLK // 2) + (t % BLK),
            N // 2 + (t // BLK) * (BLK // 2) + (t % BLK) - (BLK // 2),
        )

        # scatter slots in GEMM-row order (within-half); pads -> trash rows
        idxS = np.zeros(NMSG, dtype=np.int64)
        for h in range(2):
            lo, hi = halves[h]["row_lo"], halves[h]["row_hi"]
            idxS[lo:hi] = SENDH + np.arange(hi - lo)      # default: trash
        idxS[msg_row[cm]] = send_slot[cm]

        # one-hot matrices against the strided recv-load layout
        S = np.zeros((NSUB, J2M + J2O, P, P), dtype=np.float32)
        dm = np.nonzero(peer == c)[0]
        for m in dm:
            k = sub[m]
            d = (dest[m] - c * BLK) % P
            if of_pos[m] < 0:
                pos = send_slot[m] - c * SEG - ksub[m] * R1M
                rr = owner[m] * R1M + pos
                S[k, rr % J2M, rr // J2M, d] = 1.0
            else:
                rr2 = owner[m] * OVER + of_pos[m]
                S[k, J2M + rr2 % J2O, rr2 // J2O, d] = 1.0

        CT = np.zeros((1 + TWO_R, BLK), dtype=np.float32)
        CT[0, :] = 1.0
        for m in dm:
            CT[1 + r2[m], dest[m] - c * BLK] += 1.0

        cores.append(
            dict(
                idxA=_pack_idx16(idxA),
                idxA2=_pack_idx16(idxA2),
                idxS=_pack_idx16(idxS),
                S=S.reshape(NSUB * (J2M + J2O) * P, P).astype(BF16),
                CT=CT.astype(BF16),
            )
        )

    return dict(
        MT=MT, MTH=MTH, tile_slot=tile_slot, NMSG=NMSG, R1M=R1M, OVER=OVER,
        SEG=SEG, SENDH=SENDH, J2M=J2M, J2O=J2O, NCHUNK=NCHUNK, halves=halves,
        cores=cores,
    )


def _build(MT, MTH, tile_slot, NMSG, R1M, OVER, SEG, SENDH, J2M, J2O, NCHUNK,
           halves):
    nc = bacc.Bacc(
        "TRN2",
        target_bir_lowering=False,
        debug=False,
        enable_asserts=True,
        num_devices=NCORES,
        num_swdge_queues=4,
    )
    dt = mybir.dt

    x0 = nc.dram_tensor("x0", [N, D], dt.bfloat16, kind="ExternalInput")
    x_own = nc.dram_tensor("x_own", [BLK, D], dt.bfloat16, kind="ExternalInput")
    wrel = nc.dram_tensor("wrel", [L, RPC, D, D], dt.bfloat16, kind="ExternalInput")
    wselfT = nc.dram_tensor("wselfT", [L, D, D], dt.bfloat16, kind="ExternalInput")
    bias = nc.dram_tensor("bias", [L, 1 + TWO_R, D], dt.bfloat16, kind="ExternalInput")
    ct = nc.dram_tensor("ct", [1 + TWO_R, BLK], dt.bfloat16, kind="ExternalInput")
    wffT = nc.dram_tensor("wffT", [D, OUT], dt.bfloat16, kind="ExternalInput")
    bff = nc.dram_tensor("bff", [1, OUT], dt.bfloat16, kind="ExternalInput")
    ident = nc.dram_tensor("ident", [P, P], dt.bfloat16, kind="ExternalInput")
    idxA = nc.dram_tensor("idxA", [P, NMSG // 16], dt.int16, kind="ExternalInput")
    idxA2 = nc.dram_tensor("idxA2", [P, NMSG // 16], dt.int16, kind="ExternalInput")
    idxS = nc.dram_tensor("idxS", [P, NMSG // 16], dt.int16, kind="ExternalInput")
    s_in = nc.dram_tensor("s", [NCHUNK * P, P], dt.bfloat16, kind="ExternalInput")
    y = nc.dram_tensor("y", [BLK, OUT], dt.float32, kind="ExternalOutput")

    h_own = [
        nc.dram_tensor(f"h_own{h}", [BLK // 2, D], dt.bfloat16) for h in range(2)
    ]
    h_full = nc.dram_tensor("h_full", [N, D], dt.bfloat16, addr_space="Shared")
    send = [
        [
            nc.dram_tensor(f"send{ll}_{h}", [SENDH + MTH[h] * P, D], dt.bfloat16)
            for h in range(2)
        ]
        for ll in range(L)
    ]
    recv = [
        [nc.dram_tensor(f"recv{ll}_{h}", [SENDH, D], dt.bfloat16) for h in range(2)]
        for ll in range(L)
    ]

    Relu = mybir.ActivationFunctionType.Relu
    RG = [list(range(NCORES))]

    # SWDGE queue assignment
    Q_G0 = 0      # layer-0 gathers (plain, fired at startup)
    Q_S0 = 1      # layer-0 scatters (prep+trigger, batch per half)
    Q_G1 = 2      # layer-1 gathers (prep early, trigger on h_full)
    Q_S1 = 3      # layer-1 scatters (prep+trigger, batch per half)

    def half_chunks(h):
        lo, hi = halves[h]["row_lo"], halves[h]["row_hi"]
        out = []
        ci = 0
        for clo in range(lo, hi, CH_IDX):
            out.append((ci, clo, min(clo + CH_IDX, hi)))
            ci += 1
        return out

    with tile.TileContext(nc) as tc:
        with (
            tc.tile_pool(name="const", bufs=1) as const,
            tc.tile_pool(name="xtc", bufs=1) as xtcp,
            tc.tile_pool(name="xself", bufs=1) as xsp,
            tc.tile_pool(name="mso", bufs=1) as msop,
            tc.tile_pool(name="msgb", bufs=4) as msgbp,
            tc.tile_pool(name="selfb", bufs=8) as selfbp,
            tc.tile_pool(name="selfb1", bufs=8) as selfb1p,
            tc.tile_pool(name="hT", bufs=2) as hTp,
            tc.tile_pool(name="h", bufs=3) as hp,
            tc.tile_pool(name="psum_m", bufs=3, space="PSUM") as psum_m,
            tc.tile_pool(name="psum_o", bufs=2, space="PSUM") as psum_o,
            tc.tile_pool(name="psum_y", bufs=1, space="PSUM") as psum_y,
            tc.tile_pool(name="psum_tr", bufs=1, space="PSUM") as psum_tr,
        ):
            # ---- startup loads, ordered by need-time per queue ----
            idxA_sb = const.tile([P, NMSG // 16], dt.int16)
            nc.scalar.dma_start(idxA_sb[:], idxA.ap())
            idxS_sb = const.tile([P, NMSG // 16], dt.int16)
            nc.scalar.dma_start(idxS_sb[:], idxS.ap())

            zero_sb = const.tile([P, 4, D], dt.bfloat16)
            nc.vector.memset(zero_sb[:], 0.0)
            ones_sb = const.tile([1, P], dt.bfloat16)
            nc.vector.memset(ones_sb[:], 1.0)

            # warm-up A2A (64KB): pay ncfw cold-start during the startup
            # barrier so the first real A2A runs at warm speed.
            warm_in = nc.dram_tensor("warm_in", [128, 256], dt.bfloat16)
            warm_out = nc.dram_tensor("warm_out", [128, 256], dt.bfloat16)
            nc.sync.dma_start(warm_in.ap(), zero_sb[:, 0, :256])
            nc.gpsimd.collective_compute(
                "AllToAll",
                mybir.AluOpType.bypass,
                replica_groups=RG,
                ins=[warm_in.ap()],
                outs=[warm_out.ap()],
            )

            def zero_wire(ll, h):
                zrows = P * 4
                for lo in range(0, SENDH, zrows):
                    hi = min(lo + zrows, SENDH)
                    nc.scalar.dma_start(
                        send[ll][h].ap()[lo:hi, :],
                        zero_sb[:, : (hi - lo) // P, :],
                    )

            ct_sb = const.tile([1 + TWO_R, BLK], dt.bfloat16)
            nc.sync.dma_start(ct_sb[:], ct.ap())
            bias_sb = const.tile([1 + TWO_R, L, D], dt.bfloat16)
            nc.sync.dma_start(bias_sb[:], bias.ap().rearrange("l b d -> b l d"))
            xself0 = xsp.tile([P, KC, BLK], dt.bfloat16, tag="xself")
            nc.sync.dma_start_transpose(xself0[:], x_own.ap())
            wselfT_sb = const.tile([P, L, KC, D], dt.bfloat16)
            nc.sync.dma_start(
                wselfT_sb[:], wselfT.ap().rearrange("l (c p) n -> p l c n", p=P)
            )
            ident_sb = const.tile([P, P], dt.bfloat16)
            nc.sync.dma_start(ident_sb[:], ident.ap())
            s_sb = const.tile([P, NCHUNK, P], dt.bfloat16)
            nc.sync.dma_start(s_sb[:], s_in.ap().rearrange("(c p) n -> p c n", p=P))
            wrel_sb = [[None] * RPC for _ in range(L)]
            for ss in range(RPC):
                wt = const.tile([P, KC, D], dt.bfloat16, tag=f"wrel0_{ss}")
                nc.scalar.dma_start(
                    wt[:], wrel.ap()[0, ss].rearrange("(c p) n -> p c n", p=P)
                )
                wrel_sb[0][ss] = wt
            zero_wire(0, 0)
            zero_wire(0, 1)
            idxA2_sb = const.tile([P, NMSG // 16], dt.int16)
            nc.scalar.dma_start(idxA2_sb[:], idxA2.ap())

            # ---- layer-0 gathers: plain SWDGE on q0, data (x0) is resident ----
            def make_xc(h, ci, nrow):
                xc = xtcp.tile(
                    [P, KC, nrow], dt.bfloat16, tag=f"xc{h}_{ci}", bufs=1,
                    name=f"xc{h}_{ci}",
                )
                return xc

            xc0 = {}
            for h in range(2):
                for ci, clo, chi in half_chunks(h):
                    xc = make_xc(h, ci, chi - clo)
                    nc.gpsimd.dma_gather(
                        out_ap=xc[:],
                        in_ap=x0.ap(),
                        idxs_ap=idxA_sb[:, clo // 16 : chi // 16],
                        num_idxs=chi - clo,
                        num_idxs_reg=chi - clo,
                        elem_size=D,
                        transpose=True,
                        queue_num=Q_G0,
                    )
                    xc0[(h, ci)] = xc

            # ---- mso output tiles + layer-0 h0 scatter prep (q1) ----
            mso = [
                msop.tile(
                    [P, MTH[h], D], dt.bfloat16, tag=f"mso{h}", bufs=1,
                    name=f"mso{h}",
                )
                for h in range(2)
            ]
            def scatter(layer, h, q):
                lo, hi = halves[h]["row_lo"], halves[h]["row_hi"]
                nc.gpsimd.dma_scatter_add(
                    send[layer][h].ap(),
                    mso[h][:],
                    idxS_sb[:, lo // 16 : hi // 16],
                    hi - lo,
                    hi - lo,
                    D,
                    queue_num=q,
                )

            # ---- layer-0 self+bias tiles (fp32, added on DVE in accum) -----
            selfb0 = [None] * NSUB
            selfb1 = [None] * NSUB
            hT = [None] * NSUB

            def selfb0_compute(ks):
                for k in ks:
                    pm = psum_m.tile([P, D], dt.float32, space="PSUM", tag="pmsg")
                    for kc in range(KC):
                        nc.tensor.matmul(
                            out=pm[:],
                            lhsT=xself0[:, kc, k * P : (k + 1) * P],
                            rhs=wselfT_sb[:, 0, kc, :],
                            start=(kc == 0),
                            stop=False,
                        )
                    nc.tensor.matmul(
                        out=pm[:],
                        lhsT=ct_sb[:, k * P : (k + 1) * P],
                        rhs=bias_sb[:, 0, :],
                        start=False,
                        stop=True,
                    )
                    sb = selfbp.tile([P, D], dt.float32, tag="selfb")
                    nc.vector.tensor_copy(sb[:], pm[:])
                    selfb0[k] = sb

            def msg_gemms(layer, h, xcs):
                t0, t1 = halves[h]["tile_lo"], halves[h]["tile_hi"]
                for mt in range(t0, t1):
                    tih = mt - t0
                    ci, off = (tih * P) // CH_IDX, (tih * P) % CH_IDX
                    xc = xcs[(h, ci)]
                    ss = tile_slot[mt]
                    pm = psum_m.tile([P, D], dt.float32, space="PSUM", tag="pmsg")
                    for kc in range(KC):
                        nc.tensor.matmul(
                            out=pm[:],
                            lhsT=xc[:, kc, off : off + P],
                            rhs=wrel_sb[layer][ss][:, kc, :],
                            start=(kc == 0),
                            stop=(kc == KC - 1),
                        )
                    nc.vector.tensor_copy(mso[h][:, tih, :], pm[:])

            def a2a(layer, h):
                nc.gpsimd.collective_compute(
                    "AllToAll",
                    mybir.AluOpType.bypass,
                    replica_groups=RG,
                    ins=[send[layer][h].ap()[:SENDH, :]],
                    outs=[recv[layer][h].ap()],
                )

            # ================= layer 0 message phase =================
            selfb0_compute(range(NSUB))
            msg_gemms(0, 0, xc0)
            scatter(0, 0, Q_S0)
            a2a(0, 0)
            msg_gemms(0, 1, xc0)
            scatter(0, 1, Q_S0)
            a2a(0, 1)

            # layer-1-only consts (vector queue) + wire zeroing.  wrel layer-1
            # reuses the layer-0 weight buffers (WAR: load waits l0 GEMMs).
            for ss in range(RPC):
                wt = const.tile([P, KC, D], dt.bfloat16, tag=f"wrel0_{ss}")
                nc.scalar.dma_start(
                    wt[:], wrel.ap()[1, ss].rearrange("(c p) n -> p c n", p=P)
                )
                wrel_sb[1][ss] = wt
            wffT_sb = const.tile([P, KC, OUT], dt.bfloat16)
            nc.scalar.dma_start(
                wffT_sb[:], wffT.ap().rearrange("(c p) n -> p c n", p=P)
            )
            bff_sb = const.tile([1, OUT], dt.bfloat16)
            nc.scalar.dma_start(bff_sb[:], bff.ap())
            zero_wire(1, 0)
            zero_wire(1, 1)


            def load_recv(layer, hh):
                seg = recv[layer][hh].ap().rearrange("(s g) d -> s g d", s=NCORES)
                ov = msgbp.tile([P, J2O, D], dt.bfloat16, tag="msgO", bufs=2)
                nc.scalar.dma_start(
                    ov[:], seg[:, HSUB * R1M : HSUB * R1M + OVER, :]
                )
                mbs = []
                for kl in range(HSUB):
                    mb = msgbp.tile([P, J2M, D], dt.bfloat16, tag="msgB")
                    nc.scalar.dma_start(
                        mb[:], seg[:, kl * R1M : (kl + 1) * R1M, :]
                    )
                    mbs.append(mb)
                return ov, mbs

            JT = J2M + J2O

            def accum_half_l0(hh):
                ov, mbs = load_recv(0, hh)
                for kl in range(HSUB):
                    k = hh * HSUB + kl
                    mb = mbs[kl]
                    po = psum_o.tile([P, D], dt.float32, space="PSUM", tag="pout")
                    for j in range(J2M):
                        nc.tensor.matmul(
                            out=po[:],
                            lhsT=s_sb[:, k * JT + j, :],
                            rhs=mb[:, j, :],
                            start=(j == 0),
                            stop=False,
                        )
                    for j in range(J2O):
                        nc.tensor.matmul(
                            out=po[:],
                            lhsT=s_sb[:, k * JT + J2M + j, :],
                            rhs=ov[:, j, :],
                            start=False,
                            stop=(j == J2O - 1),
                        )
                    nc.vector.tensor_add(out=po[:], in0=po[:], in1=selfb0[k][:])
                    hsb = hp.tile([P, D], dt.bfloat16, tag="hsb")
                    nc.scalar.activation(hsb[:], po[:], Relu)
                    nc.scalar.dma_start(
                        h_own[hh].ap()[kl * P : (kl + 1) * P, :], hsb[:]
                    )
                    # h^T (PE transpose) for the direct layer-1 self GEMM
                    ptr = psum_tr.tile([P, KC, P], dt.bfloat16, space="PSUM", tag="ptr")
                    for kc in range(KC):
                        nc.tensor.transpose(
                            ptr[:, kc, :], hsb[:, kc * P : (kc + 1) * P], ident_sb[:]
                        )
                    ht = hTp.tile([P, KC, P], dt.bfloat16, tag="hT")
                    nc.vector.tensor_copy(ht[:], ptr[:])
                    hT[k] = ht
                    # layer-1 self+bias tile for sub-block k (direct layout)
                    ps = psum_m.tile([P, D], dt.float32, space="PSUM", tag="pmsg")
                    for cc in range(KC):
                        nc.tensor.matmul(
                            out=ps[:],
                            lhsT=ht[:, cc, :],
                            rhs=wselfT_sb[:, 1, cc, :],
                            start=(cc == 0),
                            stop=False,
                        )
                    nc.tensor.matmul(
                        out=ps[:],
                        lhsT=ct_sb[:, k * P : (k + 1) * P],
                        rhs=bias_sb[:, 1, :],
                        start=False,
                        stop=True,
                    )
                    sb1 = selfb1p.tile([P, D], dt.float32, tag="selfb1")
                    nc.vector.tensor_copy(sb1[:], ps[:])
                    selfb1[k] = sb1
                # AllGather of this half right away
                nc.gpsimd.collective_compute(
                    "AllGather",
                    mybir.AluOpType.bypass,
                    replica_groups=RG,
                    ins=[h_own[hh].ap()],
                    outs=[h_full.ap()[hh * (N // 2) : (hh + 1) * (N // 2), :]],
                )

            def accum_half_l1(hh):
                ov, mbs = load_recv(1, hh)
                for kl in range(HSUB):
                    k = hh * HSUB + kl
                    mb = mbs[kl]
                    po = psum_o.tile([P, D], dt.float32, space="PSUM", tag="pout")
                    for j in range(J2M):
                        nc.tensor.matmul(
                            out=po[:],
                            lhsT=s_sb[:, k * JT + j, :],
                            rhs=mb[:, j, :],
                            start=(j == 0),
                            stop=False,
                        )
                    for j in range(J2O):
                        nc.tensor.matmul(
                            out=po[:],
                            lhsT=s_sb[:, k * JT + J2M + j, :],
                            rhs=ov[:, j, :],
                            start=False,
                            stop=(j == J2O - 1),
                        )
                    nc.vector.tensor_add(out=po[:], in0=po[:], in1=selfb1[k][:])
                    h2 = hp.tile([P, D], dt.bfloat16, tag="h2")
                    nc.scalar.activation(h2[:], po[:], Relu)
                    ptr = psum_tr.tile([P, KC, P], dt.bfloat16, space="PSUM", tag="ptr")
                    for kc in range(KC):
                        nc.tensor.transpose(
                            ptr[:, kc, :], h2[:, kc * P : (kc + 1) * P], ident_sb[:]
                        )
                    h2t = hp.tile([P, KC, P], dt.bfloat16, tag="h2t")
                    nc.vector.tensor_copy(h2t[:], ptr[:])
                    # fused FF GEMM off h2^T
                    py_ = psum_y.tile([P, OUT], dt.float32, space="PSUM", tag="py")
                    for kc in range(KC):
                        nc.tensor.matmul(
                            out=py_[:],
                            lhsT=h2t[:, kc, :],
                            rhs=wffT_sb[:, kc, :],
                            start=(kc == 0),
                            stop=False,
                        )
                    nc.tensor.matmul(
                        out=py_[:], lhsT=ones_sb[:], rhs=bff_sb[:],
                        start=False, stop=True,
                    )
                    ysb = hp.tile([P, OUT], dt.float32, tag="ysb")
                    nc.vector.tensor_copy(ysb[:], py_[:])
                    nc.scalar.dma_start(y.ap()[k * P : (k + 1) * P, :], ysb[:])

            # ================= layer 0 accumulation =================
            accum_half_l0(0)
            accum_half_l0(1)

            # ================= layer 1 =================
            # plain gathers from h_full (wait the AllGathers), queues 2/0
            xc1 = {}
            qi = 0
            for h in range(2):
                for ci, clo, chi in half_chunks(h):
                    xc = make_xc(h, ci, chi - clo)
                    nc.gpsimd.dma_gather(
                        out_ap=xc[:],
                        in_ap=h_full.ap(),
                        idxs_ap=idxA2_sb[:, clo // 16 : chi // 16],
                        num_idxs=chi - clo,
                        num_idxs_reg=chi - clo,
                        elem_size=D,
                        transpose=True,
                        queue_num=(Q_G1 if qi % 2 == 0 else Q_G0),
                    )
                    qi += 1
                    xc1[(h, ci)] = xc
            msg_gemms(1, 0, xc1)
            scatter(1, 0, Q_S1)
            a2a(1, 0)
            msg_gemms(1, 1, xc1)
            scatter(1, 1, Q_S1)
            a2a(1, 1)

            accum_half_l1(0)
            accum_half_l1(1)

    nc.compile()
    return nc


def _in_maps(plan, x, W_self, b_self, W_rel, b_rel, W_ff, b_ff):
    x0 = x.astype(BF16)
    wselfT = np.ascontiguousarray(W_self.transpose(0, 2, 1)).astype(BF16)
    bias = np.concatenate([b_self[:, None, :], b_rel], axis=1).astype(BF16)
    wffT = np.ascontiguousarray(W_ff.T).astype(BF16)
    bffr = b_ff.reshape(1, OUT).astype(BF16)
    wrelT_all = np.ascontiguousarray(W_rel.transpose(0, 1, 3, 2)).astype(BF16)
    identm = np.eye(P, dtype=BF16)

    in_maps = []
    for c in range(NCORES):
        t = plan["cores"][c]
        in_maps.append(
            {
                "x0": x0,
                "x_own": np.ascontiguousarray(x0[c * BLK : (c + 1) * BLK]),
                "wrel": np.ascontiguousarray(wrelT_all[:, c * RPC : (c + 1) * RPC]),
                "wselfT": wselfT,
                "bias": bias,
                "ct": t["CT"],
                "wffT": wffT,
                "bff": bffr,
                "ident": identm,
                "idxA": t["idxA"],
                "idxA2": t["idxA2"],
                "idxS": t["idxS"],
                "s": t["S"],
            }
        )
    return in_maps


def kernel(x, dep_idx, rel_idx, gov_idx, W_self, b_self, W_rel, b_rel, W_ff, b_ff):
    global LAST_EXEC_TIME_NS, LAST_RESULTS

    x = np.asarray(x)
    dep_idx = np.asarray(dep_idx)
    rel_idx = np.asarray(rel_idx)
    gov_idx = np.asarray(gov_idx)
    W_self = np.asarray(W_self)
    b_self = np.asarray(b_self)
    W_rel = np.asarray(W_rel)
    b_rel = np.asarray(b_rel)
    W_ff = np.asarray(W_ff)
    b_ff = np.asarray(b_ff)
    assert x.shape == (N, D) and W_rel.shape == (L, TWO_R, D, D)

    key = (dep_idx.tobytes(), rel_idx.tobytes(), gov_idx.tobytes())
    if key in _CACHE:
        nc, plan = _CACHE[key]
    else:
        plan = _plan(dep_idx, rel_idx, gov_idx)
        nc = _build(
            plan["MT"], plan["MTH"], plan["tile_slot"], plan["NMSG"], plan["R1M"],
            plan["OVER"], plan["SEG"], plan["SENDH"], plan["J2M"], plan["J2O"],
            plan["NCHUNK"], plan["halves"],
        )
        _CACHE.clear()
        _CACHE[key] = (nc, plan)

    in_maps = _in_maps(plan, x, W_self, b_self, W_rel, b_rel, W_ff, b_ff)
    res = run_bass_kernel_spmd(nc, in_maps, list(range(NCORES)))
    LAST_EXEC_TIME_NS = res.exec_time_ns
    LAST_RESULTS = res
    out = np.concatenate([res.results[c]["y"] for c in range(NCORES)], axis=0)
    return out.astype(np.float32)
